# revision 1
# baseline (speedup 1.0000x reference)
"""Causal self-attention (B=4, S=2048, D=1024, H=16, HD=64) on 8 TRN2 cores.

Head-parallel sharding: core c computes heads {2c, 2c+1} end-to-end
(QKV projections, causal attention, its slice of the output projection),
returning a partial [T, D] output; the host sums the 8 partials + bo.

Note: xbar dma_start_transpose instructions are invisible to Tile's
dependency tracker (bare-AP ins/outs), so every RAW/WAR hazard around
them is pinned manually with add_dep_helper edges.
"""
import sys
sys.path.insert(0, "/opt/trn_rl_repo")
from contextlib import ExitStack

import numpy as np

import concourse.mybir as mybir
import concourse.tile as tile
from concourse import bacc
from concourse.tile_rust import add_dep_helper

FP32 = mybir.dt.float32
FP32R = mybir.dt.float32r
FP16 = mybir.dt.float16
AX = mybir.ActivationFunctionType
ALU = mybir.AluOpType

B, S, D = 4, 2048, 1024
NEG_BIG = -1.0e30
CFG = {"bufs_sc": 4, "piece": 512, "evac_out": "dve_fp16"}

_cache = {}


def _dep(a, b, reason="manual"):
    if a is not None and b is not None:
        add_dep_helper(a.ins, b.ins, True, reason)


def build_attn(Bb=B, Ss=S, Dd=D, debug=False, cfg=None):
    cfg = dict(cfg or {})
    bufs_pp = cfg.get("bufs_pp", 2)
    bufs_l = cfg.get("bufs_l", 3)
    bufs_x = cfg.get("bufs_x", 2)
    bufs_ot = cfg.get("bufs_ot", 3)
    evac_out = cfg.get("evac_out", "act_fp32")
    stop_after = cfg.get("stop_after", "full")  # qkv|softmax|transpose|pv|full
    piece = cfg.get("piece", 1024)
    bufs_pj = cfg.get("bufs_pj", 2)
    bufs_sc = cfg.get("bufs_sc", 2)
    bufs_op = cfg.get("bufs_op", 1)
    bufs_pt = cfg.get("bufs_pt", 1)
    bufs_pv = cfg.get("bufs_pv", 1)
    evac_qkv = cfg.get("evac_qkv", "act")
    dbg = set()
    if debug is True:
        dbg = {"qkv", "pn", "o"}
    elif debug:
        dbg = set(debug.split(","))
    debug = bool(dbg)
    E, HD = 128, 64
    T = Bb * Ss
    QT = 128
    NQ = Ss // QT
    DT = Dd // 128
    PIECE = piece
    SCALE = float(HD) ** 0.25

    nc = bacc.Bacc(trn_type="TRN2", debug=False, name="attn_hp")
    xT = nc.dram_tensor("xT", [Dd, T], FP32R, kind="ExternalInput")
    wqT = nc.dram_tensor("wqT", [Dd, E], FP32R, kind="ExternalInput")
    wkT = nc.dram_tensor("wkT", [Dd, E], FP32R, kind="ExternalInput")
    wvT = nc.dram_tensor("wvT", [Dd, E], FP32R, kind="ExternalInput")
    wocT = nc.dram_tensor("wocT", [E, Dd], FP32R, kind="ExternalInput")
    bq = nc.dram_tensor("bq", [E, 1], FP32, kind="ExternalInput")
    bk = nc.dram_tensor("bk", [E, 1], FP32, kind="ExternalInput")
    bv = nc.dram_tensor("bv", [E, 1], FP32, kind="ExternalInput")
    tri = nc.dram_tensor("tri", [128, 128], FP32, kind="ExternalInput")
    out_dt = FP16 if evac_out == "dve_fp16" else FP32
    out = nc.dram_tensor("out", [T, Dd], out_dt, kind="ExternalOutput")
    if debug:
        d_qT = nc.dram_tensor("d_qT", [128, Ss], FP32, kind="ExternalOutput")
        d_kT = nc.dram_tensor("d_kT", [128, Ss], FP32, kind="ExternalOutput")
        d_vT = nc.dram_tensor("d_vT", [128, Ss], FP16, kind="ExternalOutput")
        d_pn = nc.dram_tensor("d_pn", [NQ * 128, NQ * 128], FP16,
                              kind="ExternalOutput")
        d_o = nc.dram_tensor("d_o", [((NQ - 1) // 4 + 1) * 128, 512], FP32,
                             kind="ExternalOutput")

    with tile.TileContext(nc) as tc, ExitStack() as ctx:
        consts = ctx.enter_context(tc.tile_pool(name="consts", bufs=1))
        wpool = ctx.enter_context(tc.tile_pool(name="wpool", bufs=1))
        xpool = ctx.enter_context(tc.tile_pool(name="xpool", bufs=bufs_x))
        qkv = ctx.enter_context(tc.tile_pool(name="qkv", bufs=2))
        ppool = ctx.enter_context(tc.tile_pool(name="ppool", bufs=bufs_pp))
        ptpool = ctx.enter_context(tc.tile_pool(name="ptpool", bufs=bufs_pt))
        opool = ctx.enter_context(tc.tile_pool(name="opool", bufs=2))
        lpool = ctx.enter_context(tc.tile_pool(name="lpool", bufs=bufs_l))
        outp = ctx.enter_context(tc.tile_pool(name="outp", bufs=bufs_ot))
        ps_pj = ctx.enter_context(tc.tile_pool(name="ps_pj", bufs=bufs_pj, space="PSUM"))
        ps_sc = ctx.enter_context(tc.tile_pool(name="ps_sc", bufs=bufs_sc, space="PSUM"))
        ps_pv = ctx.enter_context(tc.tile_pool(name="ps_pv", bufs=bufs_pv, space="PSUM"))
        ps_op = ctx.enter_context(tc.tile_pool(name="ps_op", bufs=bufs_op, space="PSUM"))

        trit = consts.tile([128, 128], FP32)
        nc.sync.dma_start(trit[:], tri[:])
        wq_t = wpool.tile([128, DT * E], FP32R, tag="wq")
        wk_t = wpool.tile([128, DT * E], FP32R, tag="wk")
        wv_t = wpool.tile([128, DT * E], FP32R, tag="wv")
        for w_t, w_d in ((wq_t, wqT), (wk_t, wkT), (wv_t, wvT)):
            nc.sync.dma_start(
                w_t[:].rearrange("p (t e) -> p t e", e=E),
                w_d[:].rearrange("(t p) e -> p t e", p=128),
            )
        woc_t = wpool.tile([128, Dd], FP32R, tag="woc")
        nc.sync.dma_start(woc_t[:], wocT[:])
        bq_t = consts.tile([128, 1], FP32, tag="bq")
        bk_t = consts.tile([128, 1], FP32, tag="bk")
        bv_t = consts.tile([128, 1], FP32, tag="bv")
        nc.sync.dma_start(bq_t[:], bq[:])
        nc.sync.dma_start(bk_t[:], bk[:])
        nc.sync.dma_start(bv_t[:], bv[:])

        # manual-dependency state for untracked dma_start_transpose hazards
        vt_evacs = {}      # b -> list of vT evac insts
        tr_v2 = {}         # b -> v2 transpose inst
        pn_trs = {}        # (h) -> list of P' transpose insts (order of qi)
        grp_pvdone = []    # global group order -> oT2 evac inst

        for b in range(Bb):
            t0 = b * Ss
            qT_t = qkv.tile([128, Ss], FP32R, tag="qT")
            kT_t = qkv.tile([128, Ss], FP32R, tag="kT")
            vT_t = qkv.tile([128, Ss], FP16, tag="vT")
            vt_evacs[b] = []
            for ci in range(Ss // 512):
                xt = xpool.tile([128, DT * 512], FP32R, tag="xt")
                nsp = cfg.get("split_dma", 1)
                step = DT // nsp
                for si in range(nsp):
                    nc.sync.dma_start(
                        xt[:, si * step * 512:(si + 1) * step * 512].rearrange(
                            "p (t n) -> p t n", n=512),
                        xT[si * step * 128:(si + 1) * step * 128,
                           t0 + ci * 512: t0 + (ci + 1) * 512].rearrange(
                            "(t p) n -> p t n", p=128),
                    )
                cs = slice(ci * 512, (ci + 1) * 512)
                for (w_t, b_t, dst) in ((wq_t, bq_t, qT_t), (wk_t, bk_t, kT_t),
                                        (wv_t, bv_t, vT_t)):
                    pj = ps_pj.tile([128, 512], FP32, tag="pj")
                    for di in range(DT):
                        nc.tensor.matmul(
                            pj[:],
                            w_t[:, di * E:(di + 1) * E],
                            xt[:, di * 512:(di + 1) * 512],
                            start=(di == 0), stop=(di == DT - 1),
                        )
                    if evac_qkv == "act":
                        ev = nc.scalar.add(dst[:, cs], pj[:], b_t[:])
                    else:
                        ev = nc.vector.tensor_scalar_add(dst[:, cs], pj[:], b_t[:])
                    if dst is vT_t:
                        vt_evacs[b].append(ev)
                        # WAR: this vT slot may still be read by tr_v2[b-2]
                        _dep(ev, tr_v2.get(b - 2), "vT WAR")

            if "qkv" in dbg and b == 0:
                nc.sync.dma_start(d_qT[:], qT_t[:].bitcast(FP32))
                nc.sync.dma_start(d_kT[:], kT_t[:].bitcast(FP32))
                nc.sync.dma_start(d_vT[:], vT_t[:])

            v2_t = qkv.tile([128, Ss], FP16, tag="v2")
            trv = nc.scalar.dma_start_transpose(
                v2_t[:].rearrange("p (j c) -> p j c", c=128), vT_t[:])
            tr_v2[b] = trv
            for ev in vt_evacs[b]:
                _dep(trv, ev, "v2 RAW vT")
            # WAR: v2 slot (bufs=2) still read by PV of batch b-2
            if b >= 2:
                _dep(trv, grp_pvdone[(b - 1) * ((NQ - 1) // 4 + 1) - 1],
                     "v2 WAR PV")
            v2v = v2_t[:].rearrange("p (j c) -> p j c", c=128)

            for qi in range(NQ):
                if stop_after == "qkv":
                    break
                if qi % 4 == 0:
                    ptb0 = ptpool.tile([128, NQ * 512], FP16, tag="ptb0")
                    ptb1 = ptpool.tile([128, NQ * 512], FP16, tag="ptb1")
                    ptb = [ptb0, ptb1]
                    grp_trs = []
                k_len = (qi + 1) * QT
                q_sl = slice(qi * QT, (qi + 1) * QT)
                n_pieces = (k_len + PIECE - 1) // PIECE
                for h in range(2):
                    hp = slice(h * 64, (h + 1) * 64)
                    m_parts, sc_tiles = [], []
                    for pi in range(n_pieces):
                        p_lo = pi * PIECE
                        p_len = min(PIECE, k_len - p_lo)
                        sct = ps_sc.tile([128, PIECE], FP32, tag="sc")
                        for c0 in range(0, p_len, 512):
                            n = min(512, p_len - c0)
                            nc.tensor.matmul(
                                sct[:, c0:c0 + n],
                                qT_t[hp, q_sl],
                                kT_t[hp, p_lo + c0:p_lo + c0 + n],
                                start=True, stop=True,
                            )
                        if p_lo + p_len == k_len:
                            nc.vector.tensor_tensor(
                                sct[:, p_len - 128:p_len],
                                sct[:, p_len - 128:p_len], trit[:], op=ALU.add)
                        mp = lpool.tile([128, 1], FP32, tag="mp")
                        nc.vector.tensor_reduce(
                            mp[:], sct[:, 0:p_len], axis=mybir.AxisListType.X,
                            op=ALU.max)
                        m_parts.append(mp)
                        sc_tiles.append((sct, p_lo, p_len))
                    mfin = m_parts[0]
                    for k in range(1, len(m_parts)):
                        mnew = lpool.tile([128, 1], FP32, tag="mf")
                        nc.vector.tensor_tensor(
                            mnew[:], mfin[:], m_parts[k][:], op=ALU.max)
                        mfin = mnew
                    nm = lpool.tile([128, 1], FP32, tag="nm")
                    nc.vector.tensor_scalar_mul(nm[:], mfin[:], -SCALE)
                    pt = ppool.tile([128, NQ * QT], FP16, tag=f"p{h}")
                    lps, exp_insts = [], []
                    for (sct, p_lo, p_len) in sc_tiles:
                        lp = lpool.tile([128, 1], FP32, tag="lp")
                        ei = nc.scalar.activation(
                            pt[:, p_lo:p_lo + p_len], sct[:, 0:p_len], AX.Exp,
                            bias=nm[:], scale=SCALE, accum_out=lp[:])
                        lps.append(lp)
                        exp_insts.append(ei)
                    lfin = lps[0]
                    for k in range(1, len(lps)):
                        lnew = lpool.tile([128, 1], FP32, tag="lf")
                        nc.vector.tensor_tensor(
                            lnew[:], lfin[:], lps[k][:], op=ALU.add)
                        lfin = lnew
                    if stop_after == "softmax":
                        continue
                    if cfg.get("skip_norm"):
                        hist = pn_trs.setdefault(h, [])
                        if len(hist) >= bufs_pp:
                            for ei in exp_insts:
                                _dep(ei, hist[-bufs_pp], "pt WAR tr")
                        dst = ptb[h][:].rearrange("p (j g) -> p j g", g=512)[
                            :, 0:qi + 1, (qi % 4) * 128:(qi % 4) * 128 + 128]
                        tr = nc.scalar.dma_start_transpose(dst, pt[:, 0:k_len])
                        for ei in exp_insts:
                            _dep(tr, ei, "ptb RAW pt")
                        if len(grp_pvdone) >= bufs_pt:
                            _dep(tr, grp_pvdone[-bufs_pt], "ptb WAR PV")
                        hist.append(tr)
                        grp_trs.append(tr)
                        continue
                    rl = lpool.tile([128, 1], FP32, tag="rl")
                    nc.vector.reciprocal(rl[:], lfin[:])
                    pn = ppool.tile([128, NQ * QT], FP16, tag=f"pn{h}")
                    gp = nc.gpsimd.tensor_scalar_mul(
                        pn[:, 0:k_len], pt[:, 0:k_len], rl[:])
                    # WAR: pn slot (bufs=2) still read by transpose 2 back
                    hist = pn_trs.setdefault(h, [])
                    if len(hist) >= 2:
                        _dep(gp, hist[-2], "pn WAR tr")
                    if "pn" in dbg and b == 0 and h == 0:
                        nc.sync.dma_start(
                            d_pn[qi * 128:(qi + 1) * 128, 0:k_len],
                            pn[:, 0:k_len])
                    dst = ptb[h][:].rearrange("p (j g) -> p j g", g=512)[
                        :, 0:qi + 1, (qi % 4) * 128:(qi % 4) * 128 + 128]
                    tr = nc.scalar.dma_start_transpose(dst, pn[:, 0:k_len])
                    _dep(tr, gp, "ptb RAW pn")
                    # WAR: ptb slot reused bufs_pt groups back
                    if len(grp_pvdone) >= bufs_pt:
                        _dep(tr, grp_pvdone[-bufs_pt], "ptb WAR PV")
                    hist.append(tr)
                    grp_trs.append(tr)

                if stop_after in ("softmax", "transpose"):
                    continue
                if (qi % 4 == 3) or (qi == NQ - 1):
                    g = qi // 4
                    g0 = g * 4
                    W = (qi - g0 + 1) * 128
                    join = nc.sync.nop()
                    for t_i in grp_trs:
                        _dep(join, t_i, "join trs")
                    _dep(join, trv, "join v2")
                    po = ps_pv.tile([128, 512], FP32, tag="po")
                    for j in range(qi + 1):
                        lo = max(0, (j - g0) * 128)
                        for h in range(2):
                            mm = nc.tensor.matmul(
                                po[h * 64:(h + 1) * 64, lo:W],
                                v2v[:, j, h * 64:(h + 1) * 64],
                                ptb[h][:].rearrange("p (j g) -> p j g", g=512)[
                                    :, j, lo:W],
                                start=(j == 0), stop=(j == qi),
                                tile_position=(0, h * 64),
                            )
                            if j == 0:
                                _dep(mm, join, "PV RAW trs")
                    oT2 = opool.tile([128, 512], FP32R, tag="oT2")
                    ev2 = nc.scalar.copy(oT2[:, 0:W], po[:, 0:W])
                    grp_pvdone.append(ev2)
                    if stop_after == "pv":
                        continue
                    if "o" in dbg and b == 0:
                        nc.sync.dma_start(
                            d_o[g * 128:(g + 1) * 128, 0:W],
                            oT2[:, 0:W].bitcast(FP32))
                    for gi in range(g0, qi + 1):
                        ot = outp.tile([128, Dd], out_dt, tag="ot")
                        for n0 in range(0, Dd, 512):
                            n = min(512, Dd - n0)
                            op_ps = ps_op.tile([128, 512], FP32, tag="op")
                            nc.tensor.matmul(
                                op_ps[:, 0:n],
                                oT2[:, (gi - g0) * 128:(gi - g0 + 1) * 128],
                                woc_t[:, n0:n0 + n],
                                start=True, stop=True,
                            )
                            if evac_out == "dve_fp16":
                                nc.vector.tensor_copy(ot[:, n0:n0 + n], op_ps[:, 0:n])
                            else:
                                nc.scalar.copy(ot[:, n0:n0 + n], op_ps[:, 0:n])
                        nso = cfg.get("split_out", 1)
                        ostep = Dd // nso
                        for si in range(nso):
                            nc.sync.dma_start(
                                out[t0 + gi * 128: t0 + (gi + 1) * 128,
                                    si * ostep:(si + 1) * ostep],
                                ot[:, si * ostep:(si + 1) * ostep])

    nc.compile()
    return nc


def _shard(Wq, bq, Wk, bk, Wv, bv, Wo, core, xT, tri):
    sl = slice(core * 128, (core + 1) * 128)
    return {
        "xT": xT,
        "wqT": np.ascontiguousarray(Wq[sl, :].T),
        "wkT": np.ascontiguousarray(Wk[sl, :].T),
        "wvT": np.ascontiguousarray(Wv[sl, :].T),
        "wocT": np.ascontiguousarray(Wo[:, sl].T),
        "bq": np.ascontiguousarray(bq[sl].reshape(128, 1)),
        "bk": np.ascontiguousarray(bk[sl].reshape(128, 1)),
        "bv": np.ascontiguousarray(bv[sl].reshape(128, 1)),
        "tri": tri,
    }


def kernel(x, Wq, bq, Wk, bk, Wv, bv, Wo, bo, _trace=False, _results=None):
    x = np.asarray(x, dtype=np.float32)
    Wq, Wk, Wv, Wo = (np.asarray(w, dtype=np.float32) for w in (Wq, Wk, Wv, Wo))
    bq, bk, bv, bo = (np.asarray(v, dtype=np.float32) for v in (bq, bk, bv, bo))
    assert x.shape == (B, S, D), x.shape

    if "nc" not in _cache:
        _cache["nc"] = build_attn(cfg=CFG)
    nc = _cache["nc"]

    from concourse.bass_utils import run_bass_kernel_spmd

    xT = np.ascontiguousarray(x.reshape(B * S, D).T)
    tri = np.triu(np.full((128, 128), NEG_BIG, dtype=np.float32), 1)
    in_maps = [_shard(Wq, bq, Wk, bk, Wv, bv, Wo, c, xT, tri)
               for c in range(8)]
    res = run_bass_kernel_spmd(nc, in_maps, core_ids=list(range(8)),
                               trace=_trace)
    if _results is not None:
        _results.append(res)
    acc = res.results[0]["out"].astype(np.float32).copy()
    for c in range(1, 8):
        acc += res.results[c]["out"].astype(np.float32)
    acc += bo[None, :]
    return acc.reshape(B, S, D)

